# revision 41
# baseline (speedup 1.0000x reference)
"""Trainium2 Bass kernel for nn_ExpNegL2 (exp(-||a_n - t_n||) retrieval scores).

Full inputs: audio [32, 4096, 512] f32, text [32, 64, 512] f32.
Output: [32, 64, 4096] f32 = exp(-sqrt(2 - 2 * <normalize(text), normalize(audio)>)).

Sharding: data-parallel over batch, 4 batches per core across 8 cores.

Per-core pipeline (per 512-row audio t-block):
  HWDGE DMA loads audio fp32 in natural [t, d] layout, 8KB contiguous per
  partition (rows t0+4p..t0+4p+3 on partition p) -> fused square+accumulate
  (ScalarE activation accum_out, split with VectorE mul+reduce) gives row
  ssq per partition -> 1/||a|| = Exp(-0.5*Ln(ssq)) -> per-partition
  tensor_scalar scale normalizes rows and casts to bf16 -> one DMA xbar
  transpose per t-block to [d, (n,c,t)] bf16 -> PE matmul against the
  normalized+transposed text [d, m] (K=512 in 4 chunks, PSUM accumulate) ->
  ScalarE Ln / Exp / Exp computes exp(-sqrt(2-2s)) (sqrt = Exp(0.5*Ln),
  keeping every ACT function in the natural_log_exp table: zero 1283ns
  activation-table reloads) with the final store AP undoing the row
  permutation -> 2-batch [128, 4096] staging buffer -> contiguous 2MB DMA out.

Hardware gotchas baked in:
  - tensor_tensor_reduce crashes the device (NRT unrecoverable) - avoided.
  - all xbar transposes stay on the SP HWDGE ring; mixing transposes and
    copies across the two HWDGE rings corrupts data.
  - Sqrt and Exp never share an ACT table on trn2, hence the Ln/Exp sqrt.
"""

import os
import sys

sys.path.insert(0, "/opt/trn_rl_repo")

import contextlib

import numpy as np

import concourse.bacc as bacc
import concourse.tile as tile
from concourse import mybir
from concourse import bass_utils


def _env(name, default):
    return int(os.environ.get(name, default))

N_CORES = 8
B, T, M, D = 32, 4096, 64, 512
B_LOC = B // N_CORES          # batches per core
TB = 512                      # audio rows per t-block
NT = T // TB                  # t-blocks per batch
NSUB = TB // 128              # 128-row sub-tiles per t-block
NCH = D // 128                # 128-wide contraction chunks
# How many of the NSUB per-t-block square-reduces run on ScalarE instead of
# VectorE (load balancing between the two engines). ScalarE uses the fused
# activation(Square, accum_out=...); VectorE needs two ops (mul + reduce).
# NOTE: tensor_tensor_reduce crashes the device on this runtime - do not use.
N_ACT_SQUARES = _env("KN_ACTSQ", 2)
# 1: load audio fp32 via HWDGE, fuse the bf16 cast into the normalize-scale.
# 0: SWDGE DMA casts fp32->bf16 during the load.
F32PIPE = _env("KF32", 1)

F32 = mybir.dt.float32
BF16 = mybir.dt.bfloat16


def _body(ctx, tc, out, audio, text, repeat=1, ablate=()):
    nc = tc.nc
    # All three ACT functions live in the natural_log_exp_and_others table,
    # so the scalar engine never reloads activation tables (1283 ns each).
    # sqrt(x) is computed as Exp(0.5*Ln(x)), rsqrt(x) as Exp(-0.5*Ln(x)).
    Ln = mybir.ActivationFunctionType.Ln
    Exp = mybir.ActivationFunctionType.Exp
    Square = mybir.ActivationFunctionType.Square

    singles = ctx.enter_context(tc.tile_pool(name="singles", bufs=1))
    two = singles.tile([128, 1], F32)
    nc.vector.memset(two, 2.0)

    tx_pool = ctx.enter_context(tc.tile_pool(name="tx", bufs=_env("KB_TX", 2)))
    nat_pool = ctx.enter_context(tc.tile_pool(name="nat", bufs=_env("KB_NAT", 3)))
    at_pool = ctx.enter_context(tc.tile_pool(name="at", bufs=_env("KB_AT", 3)))
    small_pool = ctx.enter_context(
        tc.tile_pool(name="small", bufs=_env("KB_SMALL", 4)))
    post_pool = ctx.enter_context(
        tc.tile_pool(name="post", bufs=_env("KB_POST", 4)))
    ostage_pool = ctx.enter_context(
        tc.tile_pool(name="ostage", bufs=_env("KB_OST", 2)))
    psum_pool = ctx.enter_context(
        tc.tile_pool(name="psum", bufs=_env("KB_PSUM", 4), space="PSUM"))

    ostage = None
    for b in [b for _ in range(repeat) for b in range(B_LOC)]:
        # ---- text: load, l2-normalize rows, cast bf16, transpose to [d, m]
        txf = tx_pool.tile([M, D], F32)
        nc.sync.dma_start(out=txf, in_=text[b])
        t_scr = tx_pool.tile([M, D], F32)
        t_ssq = tx_pool.tile([M, 1], F32)
        nc.scalar.activation(t_scr, txf, Square, accum_out=t_ssq)
        t_ln = tx_pool.tile([M, 1], F32)
        nc.scalar.activation(t_ln, t_ssq, Ln)
        t_inv = tx_pool.tile([M, 1], F32)
        nc.scalar.activation(t_inv, t_ln, Exp, scale=-0.5)  # 1/||t||
        txn = tx_pool.tile([M, D], BF16)
        nc.vector.tensor_scalar_mul(txn, txf, t_inv)
        # one xbar call: out[d_rel, c, m] = txn[m, c*128 + d_rel]
        tnt = tx_pool.tile([128, NCH, M], BF16)
        nc.sync.dma_start(out=tnt, in_=txn, transpose=True)

        if b % 2 == 0:
            ostage = ostage_pool.tile([128, T], F32)
        po = (b % 2) * M

        for tb in range(NT):
            # ---- audio natural tiles, bf16 via DMA cast
            # (p n) tiling: partition p holds 4 consecutive audio rows
            # t = t0 + 4p + n -> the DMA reads 8KB contiguous per partition.
            # The resulting (n, p) permutation of the t axis is undone for
            # free by the matmul rhs access pattern below.
            src = audio[b, tb * TB:(tb + 1) * TB, :].rearrange(
                "(p n) d -> p n d", p=128
            )
            if F32PIPE:
                nat = nat_pool.tile([128, NSUB, D], F32)
                if "noload" in ablate:
                    nc.vector.memset(nat[:, 0, 0:1], 0.5)
                else:
                    nc.sync.dma_start(out=nat, in_=src)
            else:
                nat = nat_pool.tile([128, NSUB, D], BF16)
                if "noload" in ablate:
                    nc.vector.memset(nat[:, 0, 0:1], 0.5)
                else:
                    nc.gpsimd.dma_start(out=nat, in_=src)

            # ---- row ssq -> 1/||a|| (per-partition in natural layout).
            # One scratch per op: a shared scratch would WAW-serialize the
            # ScalarE and VectorE squares against each other.
            ssq = small_pool.tile([128, NSUB], F32)
            for n in range(NSUB) if "nonorm" not in ablate else []:
                sq_scr = small_pool.tile([128, D], BF16)
                if n < N_ACT_SQUARES:
                    nc.scalar.activation(
                        sq_scr, nat[:, n, :], Square, accum_out=ssq[:, n:n + 1]
                    )
                else:
                    nc.vector.tensor_mul(sq_scr, nat[:, n, :], nat[:, n, :])
                    nc.vector.reduce_sum(
                        ssq[:, n:n + 1], sq_scr, axis=mybir.AxisListType.X
                    )
            rs = small_pool.tile([128, NSUB], F32)
            inv = small_pool.tile([128, NSUB], F32)
            if "nonorm" in ablate:
                nc.vector.memset(inv, 1.0)
            else:
                nc.scalar.activation(rs, ssq, Ln)
                nc.scalar.activation(inv, rs, Exp, scale=-0.5)  # 1/||a||

            # ---- normalize rows (bf16, per-partition scalar)
            natn = nat_pool.tile([128, NSUB, D], BF16)
            if "noscale" in ablate:
                nc.vector.memset(natn[:, 0, 0:1], 0.5)
            else:
                for n in range(NSUB):
                    nc.vector.tensor_scalar_mul(
                        natn[:, n, :], nat[:, n, :], inv[:, n:n + 1]
                    )

            # ---- transpose to [d, t] via one DMA xbar call per t-block:
            # out[d_rel, n, c, t_sub] = natn[t_sub, n, c*128 + d_rel]
            at = at_pool.tile([128, NSUB, NCH, 128], BF16)
            if "notr" in ablate:
                nc.vector.memset(at[:, 0, 0, 0:1], 0.5)
            else:
                # keep ALL xbar transposes on one HWDGE ring (SP): running
                # transposes concurrently with copies on the other ring
                # corrupts data (xbar-mode hazard observed on HW)
                nc.sync.dma_start(out=at, in_=natn, transpose=True)

            # ---- dots = txn @ natn^T, contracted over d in 4 chunks.
            # Output column j = n*128 + p corresponds to t = t0 + 4p + n;
            # the final store AP into ostage undoes this permutation.
            dots = psum_pool.tile([M, TB], F32)
            if "nomm" in ablate:
                nc.vector.memset(dots[:, 0:1], 0.5)
            else:
                for c in range(NCH):
                    nc.tensor.matmul(
                        dots, tnt[:, c, :], at[:, :, c, :],
                        start=(c == 0), stop=(c == NCH - 1),
                    )

            # ---- exp(-sqrt(2 - 2s)); 2-2s >= ~1.5 for this data, no clamp.
            # sqrt via exp(0.5*ln(.)) keeps ACT on one activation table.
            if "nopost" in ablate:
                if tb == 0:
                    nc.vector.memset(ostage[po:po + M, 0:1], 0.5)
            else:
                lnz = post_pool.tile([M, TB], F32)
                nc.scalar.activation(lnz, dots, Ln, bias=two[:M], scale=-2.0)
                dist = post_pool.tile([M, TB], F32)
                nc.scalar.activation(dist, lnz, Exp, scale=0.5)
                # write column j=(n, p) to t-slot 4p+n: strided free AP
                odst = ostage[po:po + M, tb * TB:(tb + 1) * TB].rearrange(
                    "m (p n) -> m n p", n=NSUB
                )
                dsrc = dist.rearrange("m (n p) -> m n p", n=NSUB)
                nc.scalar.activation(odst, dsrc, Exp, scale=-1.0)

        if b % 2 == 1:
            dst = out[b - 1:b + 1].rearrange("b m t -> (b m) t")
            nc.sync.dma_start(out=dst, in_=ostage)


_NC_CACHE = {}


def _build(repeat=1):
    if repeat in _NC_CACHE:
        return _NC_CACHE[repeat]
    nc = bacc.Bacc(
        "TRN2", target_bir_lowering=False, debug=False,
        enable_asserts=False, num_devices=N_CORES,
    )
    audio = nc.dram_tensor("audio", [B_LOC, T, D], F32, kind="ExternalInput").ap()
    text = nc.dram_tensor("text", [B_LOC, M, D], F32, kind="ExternalInput").ap()
    out = nc.dram_tensor("out", [B_LOC, M, T], F32, kind="ExternalOutput").ap()
    with tile.TileContext(nc) as tc:
        with contextlib.ExitStack() as ctx:
            _body(ctx, tc, out, audio, text, repeat=repeat)
    nc.compile()
    _NC_CACHE[repeat] = nc
    return nc


def kernel(audio: np.ndarray, text: np.ndarray) -> np.ndarray:
    nc = _build()
    in_maps = []
    for i in range(N_CORES):
        sl = slice(i * B_LOC, (i + 1) * B_LOC)
        in_maps.append({
            "audio": np.ascontiguousarray(audio[sl], dtype=np.float32),
            "text": np.ascontiguousarray(text[sl], dtype=np.float32),
        })
    res = bass_utils.run_bass_kernel_spmd(nc, in_maps, core_ids=list(range(N_CORES)))
    return np.concatenate([r["out"] for r in res.results], axis=0)
